# revision 10
# baseline (speedup 1.0000x reference)
"""Trainium2 Bass kernel for nn_BilinearMatrixSAE.

Strategy (8 NeuronCores, SPMD):
- Feature-sharded bf16 encode: each core computes pre = x_flat @ M_encT for its
  4096-feature shard (M_encT tiles generated on-chip from rank-1 V,W factors).
- Per-256-chunk top-8 candidate extraction (DVE max/max_index); candidates
  packed as u32 sort keys (value-top-16-bits | feature-idx-low-16).
- AllGather of candidate keys; each core merges the 1024 candidates for its own
  128 batch rows, takes top-64 by key (max/match_replace rounds).
- Exact fp32 refinement of the 64 candidates per row: gather V/W/b rows via
  dma_gather, pre = v^T X w + b on PE/DVE in fp32 -> exact top-32 threshold.
- Decode on PE from gathered V_dec/W_dec scaled by masked coefficients; bias via
  identity-matmul; MSE partial per core.
- coeffs output: device writes dense zeros; host overlays the (val, idx) pairs
  (exact values; losers carry val=0 so overlay is harmless).
"""

import sys

sys.path.insert(0, "/opt/trn_rl_repo")

import numpy as np
import ml_dtypes

import concourse.bass as bass
import concourse.tile as tile
import concourse.mybir as mybir
from concourse import bacc
from concourse.bass_utils import run_bass_kernel_spmd
from concourse.masks import make_identity

dt = mybir.dt
bf16 = ml_dtypes.bfloat16

B, DK, DV = 1024, 64, 64
D = DK * DV  # 4096
NF = 32768
NCORES = 8
SHARD = NF // NCORES  # 4096
ROWS = B // NCORES  # 128
NFT = 8  # feature tiles of 512 within shard
FT = SHARD // NFT  # 512
NKV = D // 128  # 32 contraction chunks
CH = 256  # candidate chunk width
NCAND = 64  # refine slots per row

_NC_CACHE = {}


def build_nc():
    nc = bacc.Bacc("TRN2", target_bir_lowering=False)

    # inputs
    xT_d = nc.dram_tensor("xT", [D, B], dt.bfloat16, kind="ExternalInput")
    vt_d = nc.dram_tensor("vt", [64, SHARD], dt.bfloat16, kind="ExternalInput")
    wt2_d = nc.dram_tensor("wt2", [128, SHARD], dt.bfloat16, kind="ExternalInput")
    selbank_d = nc.dram_tensor("selbank", [64, NKV, 128], dt.bfloat16, kind="ExternalInput")
    benc_d = nc.dram_tensor("benc", [1, SHARD], dt.bfloat16, kind="ExternalInput")
    ones1_d = nc.dram_tensor("ones1", [1, 128], dt.bfloat16, kind="ExternalInput")
    xo_d = nc.dram_tensor("xo", [64, ROWS, 64], dt.float32, kind="ExternalInput")
    ident2_d = nc.dram_tensor("ident2", [128, 64], dt.float32, kind="ExternalInput")
    bias2_d = nc.dram_tensor("bias2", [128, 64], dt.float32, kind="ExternalInput")
    foff_d = nc.dram_tensor("foff", [128, 1], dt.float32, kind="ExternalInput")
    vwb_d = nc.dram_tensor("vwb", [NF, 192], dt.float32, kind="ExternalInput")
    vwd_d = nc.dram_tensor("vwd", [NF, 128], dt.float32, kind="ExternalInput")

    # outputs
    coeffs_d = nc.dram_tensor("coeffs_z", [ROWS, NF], dt.float32, kind="ExternalOutput")
    vals_d = nc.dram_tensor("vals", [ROWS, NCAND], dt.float32, kind="ExternalOutput")
    idx_d = nc.dram_tensor("idx", [ROWS, NCAND], dt.uint16, kind="ExternalOutput")
    recon_d = nc.dram_tensor("recon", [ROWS, DK, DV], dt.float32, kind="ExternalOutput")
    mse_d = nc.dram_tensor("msep", [1, 1], dt.float32, kind="ExternalOutput")

    AO = mybir.AluOpType

    with tile.TileContext(nc) as tc:
        with (
            tc.tile_pool(name="persist", bufs=1) as pp,
            tc.tile_pool(name="dram", bufs=1, space="DRAM") as dram,
        ):
            # ---------- P0: persistent loads ----------
            vt_sb = pp.tile([64, SHARD], dt.bfloat16)
            nc.sync.dma_start(vt_sb[:], vt_d[:])
            wt2_sb = pp.tile([128, SHARD], dt.bfloat16)
            nc.sync.dma_start(wt2_sb[:], wt2_d[:])
            selbank_sb = pp.tile([64, NKV, 128], dt.bfloat16)
            nc.sync.dma_start(selbank_sb[:], selbank_d[:])
            benc_sb = pp.tile([1, SHARD], dt.bfloat16)
            nc.sync.dma_start(benc_sb[:], benc_d[:])
            ones1_sb = pp.tile([1, 128], dt.bfloat16)
            nc.sync.dma_start(ones1_sb[:], ones1_d[:])
            ident2_sb = pp.tile([128, 64], dt.float32)
            nc.sync.dma_start(ident2_sb[:], ident2_d[:])
            bias2_sb = pp.tile([128, 64], dt.float32)
            nc.sync.dma_start(bias2_sb[:], bias2_d[:])
            foff_sb = pp.tile([128, 1], dt.float32)
            nc.sync.dma_start(foff_sb[:], foff_d[:])

            # coeffs zero-fill (independent of everything else)
            zer_sb = pp.tile([128, 1024], dt.float32)
            nc.vector.memset(zer_sb[:], 0.0)
            for z in range(NF // 1024):
                nc.sync.dma_start(coeffs_d[:, z * 1024 : (z + 1) * 1024], zer_sb[:])

            ckeys = pp.tile([128, NCORES, 128], dt.uint32)

            tc.strict_bb_all_engine_barrier()

            # ---------- P1: encode + candidates ----------
            with (
                tc.tile_pool(name="encp", bufs=1) as encp,
                tc.tile_pool(name="slabp", bufs=2) as slabp,
                tc.tile_pool(name="bcp", bufs=2, space="PSUM") as bcp,
                tc.tile_pool(name="bcsbp", bufs=3) as bcsbp,
                tc.tile_pool(name="prep", bufs=4, space="PSUM") as prep,
                tc.tile_pool(name="presbp", bufs=4) as presbp,
                tc.tile_pool(name="tmp1", bufs=4) as tmp1,
            ):
                xT_sb = encp.tile([128, NKV, B], dt.bfloat16)
                nc.sync.dma_start(xT_sb[:], xT_d[:].rearrange("(a p) n -> p a n", p=128))
                for ft in range(NFT):
                    fsl = slice(ft * FT, (ft + 1) * FT)
                    slab = slabp.tile([128, NKV, FT], dt.bfloat16, tag="slab")
                    for kp in range(NKV):
                        bc_ps = bcp.tile([128, FT], dt.float32, space="PSUM", tag="bc")
                        nc.tensor.matmul(
                            bc_ps[:],
                            selbank_sb[:, kp, :],
                            vt_sb[:, fsl],
                            start=True,
                            stop=True,
                        )
                        bc_sb = bcsbp.tile([128, FT], dt.bfloat16, tag="bcsb")
                        nc.scalar.activation(
                            out=bc_sb[:], in_=bc_ps[:],
                            func=mybir.ActivationFunctionType.Copy,
                        )
                        nc.vector.tensor_tensor(
                            out=slab[:, kp, :], in0=bc_sb[:], in1=wt2_sb[:, fsl],
                            op=AO.mult,
                        )
                    for bt in range(8):
                        pre_ps = prep.tile([128, FT], dt.float32, space="PSUM", tag="pre")
                        for kv in range(NKV):
                            nc.tensor.matmul(
                                pre_ps[:],
                                xT_sb[:, kv, bt * 128 : (bt + 1) * 128],
                                slab[:, kv, :],
                                start=(kv == 0),
                                stop=False,
                            )
                        nc.tensor.matmul(
                            pre_ps[:],
                            ones1_sb[:],
                            benc_sb[:, fsl],
                            start=False,
                            stop=True,
                        )
                        pre_sb = presbp.tile([128, FT], dt.float32, tag="presb")
                        nc.vector.tensor_copy(pre_sb[:], pre_ps[:])
                        for ch in range(FT // CH):
                            sl = pre_sb[:, ch * CH : (ch + 1) * CH]
                            mxt = tmp1.tile([128, 8], dt.float32, tag="mxt")
                            nc.vector.max(out=mxt[:], in_=sl)
                            mit = tmp1.tile([128, 8], dt.uint16, tag="mit")
                            nc.vector.max_index(out=mit[:], in_max=mxt[:], in_values=sl)
                            mitf = tmp1.tile([128, 8], dt.float32, tag="mitf")
                            nc.vector.tensor_copy(mitf[:], mit[:])
                            idxf = tmp1.tile([128, 8], dt.float32, tag="idxf")
                            nc.vector.tensor_scalar(
                                out=idxf[:], in0=mitf[:],
                                scalar1=foff_sb[:, 0:1],
                                scalar2=float(ft * FT + ch * CH),
                                op0=AO.add, op1=AO.add,
                            )
                            idx32 = tmp1.tile([128, 8], dt.uint32, tag="idx32")
                            nc.vector.tensor_copy(idx32[:], idxf[:])
                            kslot = (ft * (FT // CH) + ch) * 8
                            vhi = tmp1.tile([128, 8], dt.uint32, tag="vhi")
                            nc.vector.tensor_scalar(
                                out=vhi[:], in0=mxt[:].bitcast(dt.uint32),
                                scalar1=0xFFFF0000, scalar2=None,
                                op0=AO.bitwise_and,
                            )
                            nc.vector.tensor_tensor(
                                out=ckeys[:, bt, kslot : kslot + 8],
                                in0=vhi[:], in1=idx32[:], op=AO.bitwise_or,
                            )

            tc.strict_bb_all_engine_barrier()

            # ---------- P2: allgather candidates ----------
            cand_dram = dram.tile([B, 128], dt.uint32)
            nc.sync.dma_start(
                cand_dram[:].rearrange("(bt p) n -> p bt n", p=128), ckeys[:]
            )
            agout = dram.tile([NCORES, B, 128], dt.uint32, addr_space="Shared")
            nc.gpsimd.collective_compute(
                "AllGather",
                AO.bypass,
                replica_groups=[list(range(NCORES))],
                ins=[cand_dram[:].opt()],
                outs=[agout[:].opt()],
            )

            tc.strict_bb_all_engine_barrier()

            # ---------- P3: merge own rows, top-64 ----------
            with tc.tile_critical():
                pid = nc.gpsimd.partition_id()
            merged = pp.tile([128, NCORES, 128], dt.uint32)
            agflat = agout[:].rearrange("c b n -> (c b) n")
            for c in range(NCORES):
                nc.gpsimd.dma_start(
                    merged[:, c, :],
                    agflat[bass.ds(c * B + pid * ROWS, ROWS), :],
                )
            work = pp.tile([128, NCORES * 128], dt.float32)
            nc.vector.tensor_copy(
                work[:].bitcast(dt.uint32),
                merged[:].rearrange("p c n -> p (c n)"),
            )
            skeys = pp.tile([128, NCAND], dt.float32)
            for r in range(NCAND // 8):
                nc.vector.max(out=skeys[:, r * 8 : r * 8 + 8], in_=work[:])
                nc.vector.match_replace(
                    out=work[:],
                    in_to_replace=skeys[:, r * 8 : r * 8 + 8],
                    in_values=work[:],
                    imm_value=-1e30,
                )
            selidx32 = pp.tile([128, NCAND], dt.uint32)
            nc.vector.tensor_scalar(
                out=selidx32[:], in0=skeys[:].bitcast(dt.uint32),
                scalar1=0xFFFF, scalar2=None, op0=AO.bitwise_and,
            )
            selidx = pp.tile([128, NCAND], dt.uint16)
            nc.vector.tensor_copy(selidx[:], selidx32[:])
            nc.sync.dma_start(idx_d[:], selidx[:])

            # gather index list, wrapped layout: element k=b*64+j at [g*16+k%16, k//16]
            gi = pp.tile([128, 512], dt.int16)
            for pmod in range(16):
                src = selidx[:, pmod:NCAND:16].bitcast(dt.int16)  # [128, 4]
                dstv = gi[pmod : pmod + 1, :].rearrange("o (b4 c4) -> o b4 c4", c4=4)
                nc.sync.dma_start(dstv, src)
            for g in range(1, 8):
                nc.sync.dma_start(gi[g * 16 : g * 16 + 16, :], gi[0:16, :])

            tc.strict_bb_all_engine_barrier()

            # ---------- P4: gathers + xo load ----------
            xop = tc.alloc_tile_pool(name="xop", bufs=1)
            xo_sb = xop.tile([64, ROWS, 64], dt.float32)
            nc.sync.dma_start(xo_sb[:], xo_d[:])
            refp = tc.alloc_tile_pool(name="refp", bufs=1)
            vwb_g = refp.tile([128, NCAND, 192], dt.float32)
            nc.gpsimd.dma_gather(
                out_ap=vwb_g[:], in_ap=vwb_d[:], idxs_ap=gi[:],
                num_idxs=ROWS * NCAND, num_idxs_reg=ROWS * NCAND,
                elem_size=192, single_packet=False,
            )
            # W+b of encode, partition-shifted to base 0: wgb[j, h, c, 0:65]
            wgb = refp.tile([64, 2, NCAND, 65], dt.float32)
            for h in range(2):
                nc.sync.dma_start(
                    wgb[:, h, :, :], vwb_g[h * 64 : h * 64 + 64, :, 64:129]
                )
            ident128 = pp.tile([128, 128], dt.float32)
            make_identity(nc, ident128[:])

            tc.strict_bb_all_engine_barrier()

            # ---------- P5: exact refine ----------
            pre_cols = pp.tile([64, ROWS], dt.float32)
            pre_ex = pp.tile([128, NCAND], dt.float32)
            with (
                tc.tile_pool(name="tpp", bufs=2, space="PSUM") as tpp,
                tc.tile_pool(name="vgtp", bufs=3) as vgtp,
                tc.tile_pool(name="tbp", bufs=4, space="PSUM") as tbp,
                tc.tile_pool(name="junkp", bufs=4) as junkp,
            ):
                for c in range(NCAND):
                    tp_ps = tpp.tile([64, 128], dt.float32, space="PSUM", tag="tp")
                    nc.tensor.transpose(
                        out=tp_ps[:], in_=vwb_g[:, c, 0:64], identity=ident128[:]
                    )
                    vgt = vgtp.tile([64, 128], dt.float32, tag="vgt")
                    nc.vector.tensor_copy(vgt[:], tp_ps[:])
                    for h in range(2):
                        b = 2 * c + h
                        t_ps = tbp.tile([64, 64], dt.float32, space="PSUM", tag="tb")
                        nc.tensor.matmul(
                            t_ps[:],
                            vgt[:, h * 64 : h * 64 + 64],
                            xo_sb[:, b, :],
                            start=True,
                            stop=True,
                        )
                        junk = junkp.tile([64, 64], dt.float32, tag="junk")
                        nc.vector.scalar_tensor_tensor(
                            out=junk[:],
                            in0=t_ps[:],
                            scalar=1.0,
                            in1=wgb[:, h, c, 0:64],
                            op0=AO.mult,
                            op1=AO.mult,
                            accum_out=pre_cols[:, b : b + 1],
                        )
                # add b_enc: pre_cols col b=2c+h gets wgb[:, h, c, 64]
                nc.vector.tensor_tensor(
                    out=pre_cols[:].rearrange("p (c h) -> p c h", h=2),
                    in0=pre_cols[:].rearrange("p (c h) -> p c h", h=2),
                    in1=wgb[:, :, :, 64:65].rearrange("p h c o -> p c (h o)"),
                    op=AO.add,
                )
                # transpose pre_cols -> pre_ex [128 rows, 64]
                pex_ps = tpp.tile([128, 64], dt.float32, space="PSUM", tag="pex")
                nc.tensor.transpose(
                    out=pex_ps[:], in_=pre_cols[:], identity=ident2_sb[0:64, :]
                )
                nc.vector.tensor_copy(pre_ex[:], pex_ps[:])

            refp.release()
            tc.strict_bb_all_engine_barrier()

            # ---------- P6: exact top-32 threshold + masked coefficients ----------
            decp0 = tc.alloc_tile_pool(name="latep", bufs=1)
            vwd_g = decp0.tile([128, NCAND, 128], dt.float32)
            nc.gpsimd.dma_gather(
                out_ap=vwd_g[:], in_ap=vwd_d[:], idxs_ap=gi[:],
                num_idxs=ROWS * NCAND, num_idxs_reg=ROWS * NCAND,
                elem_size=128, single_packet=False,
            )
            work2 = pp.tile([128, NCAND], dt.float32)
            nc.vector.tensor_copy(work2[:], pre_ex[:])
            sv = pp.tile([128, 32], dt.float32)
            for r in range(4):
                nc.vector.max(out=sv[:, r * 8 : r * 8 + 8], in_=work2[:])
                nc.vector.match_replace(
                    out=work2[:],
                    in_to_replace=sv[:, r * 8 : r * 8 + 8],
                    in_values=work2[:],
                    imm_value=-1e30,
                )
            crel = pp.tile([128, NCAND], dt.float32)
            nc.vector.tensor_scalar_max(crel[:], pre_ex[:], 0.0)
            cvals = pp.tile([128, NCAND], dt.float32)
            nc.vector.scalar_tensor_tensor(
                out=cvals[:],
                in0=pre_ex[:],
                scalar=sv[:, 31:32],
                in1=crel[:],
                op0=AO.is_ge,
                op1=AO.mult,
            )
            nc.sync.dma_start(vals_d[:], cvals[:])

            # cval_g[h*64+j, c] = cvals[2c+h, j] : transpose then parity de-interleave
            cval_g = pp.tile([128, NCAND], dt.float32)
            cvT_sb = pp.tile([64, 128], dt.float32)
            with tc.tile_pool(name="cvp", bufs=1, space="PSUM") as cvp:
                cv_ps = cvp.tile([64, 128], dt.float32, space="PSUM")
                nc.tensor.transpose(
                    out=cv_ps[:], in_=cvals[:], identity=ident128[:]
                )
                nc.vector.tensor_copy(cvT_sb[:], cv_ps[:])
            for h in range(2):
                nc.sync.dma_start(
                    cval_g[h * 64 : h * 64 + 64, :], cvT_sb[:, h:ROWS:2]
                )

            tc.strict_bb_all_engine_barrier()

            # ---------- P7: decode + mse ----------
            cvg = decp0.tile([128, NCAND, 64], dt.float32)
            for c in range(NCAND):
                nc.vector.tensor_scalar_mul(
                    cvg[:, c, :], vwd_g[:, c, 0:64], cval_g[:, c : c + 1]
                )
            msecols = pp.tile([64, ROWS], dt.float32)
            with (
                tc.tile_pool(name="decp", bufs=6, space="PSUM") as decp,
                tc.tile_pool(name="reconp", bufs=6) as reconp,
                tc.tile_pool(name="dsbp", bufs=4) as dsbp,
            ):
                for b in range(ROWS):
                    h, c = b % 2, b // 2
                    ps_r = decp.tile([64, 64], dt.float32, space="PSUM", tag="dec")
                    nc.tensor.matmul(
                        ps_r[:],
                        cvg[h * 64 : h * 64 + 64, c, :],
                        vwd_g[h * 64 : h * 64 + 64, c, 64:128],
                        start=True,
                        stop=False,
                    )
                    nc.tensor.matmul(
                        ps_r[:],
                        ident2_sb[h * 64 : h * 64 + 64, :],
                        bias2_sb[h * 64 : h * 64 + 64, :],
                        start=False,
                        stop=True,
                    )
                    rec = reconp.tile([64, 64], dt.float32, tag="rec")
                    nc.scalar.activation(
                        out=rec[:], in_=ps_r[:],
                        func=mybir.ActivationFunctionType.Copy,
                    )
                    nc.sync.dma_start(recon_d[b, :, :], rec[:])
                    dsb = dsbp.tile([64, 64], dt.float32, tag="dsb")
                    nc.vector.tensor_sub(dsb[:], rec[:], xo_sb[:, b, :])
                    d2 = dsbp.tile([64, 64], dt.float32, tag="d2")
                    nc.scalar.activation(
                        out=d2[:], in_=dsb[:],
                        func=mybir.ActivationFunctionType.Square,
                        accum_out=msecols[:, b : b + 1],
                    )
            msered = pp.tile([64, 1], dt.float32)
            nc.vector.reduce_sum(msered[:], msecols[:], axis=mybir.AxisListType.X)
            ones64f = pp.tile([64, 1], dt.float32)
            nc.vector.memset(ones64f[:], 1.0)
            tc.strict_bb_all_engine_barrier()
            with tc.tile_pool(name="msp", bufs=1, space="PSUM") as msp:
                ms_ps = msp.tile([1, 1], dt.float32, space="PSUM")
                nc.tensor.matmul(
                    ms_ps[:], msered[:], ones64f[:], start=True, stop=True
                )
                mse_sb = pp.tile([1, 1], dt.float32)
                nc.vector.tensor_scalar_mul(
                    mse_sb[:], ms_ps[:], 1.0 / float(B * D)
                )
            nc.sync.dma_start(mse_d[:], mse_sb[:])
            decp0.release()
            xop.release()

    nc.finalize()
    return nc


def _host_prep(x, V_enc, W_enc, b_enc, V_dec, W_dec, bias):
    x = np.ascontiguousarray(x, dtype=np.float32)
    xf = x.reshape(B, D)
    xbfT = np.ascontiguousarray(xf.astype(bf16).T)  # [D, B] bf16

    V = np.ascontiguousarray(V_enc[:, 0, :], dtype=np.float32)
    W = np.ascontiguousarray(W_enc[:, 0, :], dtype=np.float32)
    Vd = np.ascontiguousarray(V_dec[:, 0, :], dtype=np.float32)
    Wd = np.ascontiguousarray(W_dec[:, 0, :], dtype=np.float32)
    b_enc = np.ascontiguousarray(b_enc, dtype=np.float32)
    bias = np.ascontiguousarray(bias, dtype=np.float32)

    vwb = np.concatenate(
        [V, W, np.repeat(b_enc[:, None], 64, axis=1)], axis=1
    ).astype(np.float32)  # [NF, 192]
    vwd = np.concatenate([Vd, Wd], axis=1).astype(np.float32)  # [NF, 128]

    selbank = np.zeros((64, NKV, 128), dtype=bf16)
    for kp in range(NKV):
        for p in range(128):
            selbank[2 * kp + p // 64, kp, p] = 1.0

    ident2 = np.vstack([np.eye(64, dtype=np.float32)] * 2)
    bias2 = np.vstack([bias, bias]).astype(np.float32)
    ones1 = np.ones((1, 128), dtype=bf16)

    in_maps = []
    for c in range(NCORES):
        sh = slice(c * SHARD, (c + 1) * SHARD)
        rows = slice(c * ROWS, (c + 1) * ROWS)
        xo = np.ascontiguousarray(
            x[rows].reshape(ROWS, 64, 64).transpose(1, 0, 2)
        ).astype(np.float32)
        in_maps.append(
            {
                "xT": xbfT,
                "vt": np.ascontiguousarray(V[sh].T).astype(bf16),
                "wt2": np.vstack([W[sh].T, W[sh].T]).astype(bf16),
                "selbank": selbank,
                "benc": b_enc[sh][None, :].astype(bf16),
                "ones1": ones1,
                "xo": xo,
                "ident2": ident2,
                "bias2": bias2,
                "foff": np.full((128, 1), float(c * SHARD), dtype=np.float32),
                "vwb": vwb,
                "vwd": vwd,
            }
        )
    return in_maps


def kernel(x, V_enc, W_enc, b_enc, V_dec, W_dec, bias, _want_time=False):
    if "nc" not in _NC_CACHE:
        _NC_CACHE["nc"] = build_nc()
    nc = _NC_CACHE["nc"]

    in_maps = _host_prep(
        np.asarray(x), np.asarray(V_enc), np.asarray(W_enc), np.asarray(b_enc),
        np.asarray(V_dec), np.asarray(W_dec), np.asarray(bias),
    )
    res = run_bass_kernel_spmd(nc, in_maps, core_ids=list(range(NCORES)))

    recon = np.concatenate([r["recon"] for r in res.results], axis=0)
    coeffs = np.concatenate([r["coeffs_z"] for r in res.results], axis=0)
    for c, r in enumerate(res.results):
        rows = slice(c * ROWS, (c + 1) * ROWS)
        idx = r["idx"].astype(np.int64)
        vals = r["vals"]
        np.save(f"/tmp/dbg_vals_{c}.npy", vals)
        np.save(f"/tmp/dbg_idx_{c}.npy", r["idx"])
        rr, jj = np.nonzero(vals)
        bad = idx[rr, jj] >= NF
        if bad.any():
            import sys as _s
            print(f"core {c}: {bad.sum()} bad winner idx; sample rows {rr[bad][:5]}, idx {idx[rr, jj][bad][:5]}, vals {vals[rr, jj][bad][:5]}", file=_s.stderr)
        ok = ~bad
        coeffs[c * ROWS + rr[ok], idx[rr, jj][ok]] = vals[rr, jj][ok]
    mse = np.float32(sum(float(r["msep"][0, 0]) for r in res.results))

    if _want_time:
        return (recon, coeffs, mse), res
    return recon, coeffs, mse
